# revision 13
# baseline (speedup 1.0000x reference)
"""Trainium2 Bass kernel for the 2-layer GATv2 + MLP-head model (nn_GAT_21028159881586).

Strategy (8 NeuronCores, SPMD single NEFF):
  * Destination-block partitioning: global nodes are split into 8 slices of
    3750 (padded to 3840 = 30 windows x 128 per core).  Core c owns all edges
    whose destination lands in its slice, so segment softmax + aggregation are
    core-local.
  * Per layer: data-parallel node transforms xl = x@Wl+bl / xr = x@Wr+br on
    the local slice; xr stays resident in SBUF, xl is AllGathered across the
    8 cores, then 30 windows of 128 destinations each are processed.
  * Per window (the V2 pipeline -- exactly ONE row-major gather stream):
      - dma_gather of xl rows (by edge source) in (edge, channel) layout only
        (3 chunks of 896 idxs).  The (channel, edge) copy and the xr[dst]
        gather of V1 are gone: xr[dst] is reconstructed on the PE from the
        128 window xr rows via the transposed 0/1 scatter matrix, and the
        logits contract over channels on the DVE instead of the PE.
      - S   [e,d] = (drl[e] == d)   built by one DVE compare  (agg lhsT)
        S^T [d,e]                   built by DVE compare against a partition-
        broadcast of drl (broadcast done by a stride-0 DMA or a K=1 matmul)
      - per tile t: PSUM m = S^T.T @ xr_window + I.T @ xl_src  (PE), then
        ACT Prelu(m) -> lr, DVE lr *= att (broadcast), DVE segmented reduce
        -> logits, += pad bias, ACT exp, DVE xl_src *= exp (in place), PE
        aggregation matmuls into PSUM [agg | den].
      - normalize by 1/den, add bias, ELU, write the 128 output rows.
  * Softmax max-subtraction is skipped (logits are O(1); exp cannot overflow).
  * MLP head: batch rows are assigned to the core owning their var node, the
    selected h2 rows are dma_gathered transposed, and the 3-layer MLP runs
    fully transposed.

Everything runs in fp16 with fp32 PSUM accumulation.
"""

import numpy as np

import concourse.bacc as bacc
import concourse.tile as tile
import concourse.mybir as mybir
from concourse.bass_utils import run_bass_kernel_spmd

P = 128
NCORES = 8
N = 30000
NLOC_REAL = 3750          # real nodes per core
WIN = 30                  # destination windows per core
NLOC = WIN * P            # 3840 padded nodes per core
NALL = NCORES * NLOC      # 30720 padded global nodes
IN_DIM = 1281
KCH = 11                  # input-dim chunks of 128
KPAD = KCH * P            # 1408
HID = 256
HEADS1 = 4
BLOC = 640                # padded batch rows per core (actual max ~554)
NEG = 0.2
PAD_BIAS = -30000.0
BATCH_M = 4               # tiles per PSUM m batch

f32 = mybir.dt.float32
f16 = mybir.dt.float16
i16 = mybir.dt.int16
AF = mybir.ActivationFunctionType
OP = mybir.AluOpType

USE_BCAST_DMA = True      # stride-0 partition broadcast via DMA for drl row

_nc_cache = {}


def _wrap16(idx2d: np.ndarray) -> np.ndarray:
    """(W, E) int -> (W*128, E//16) int16, wrapped in 16 partitions, replicated
    across the 8 gpsimd cores."""
    w, e = idx2d.shape
    assert e % 16 == 0
    t = idx2d.reshape(w, e // 16, 16).transpose(0, 2, 1)       # (W, 16, E/16)
    return np.tile(t, (1, 8, 1)).reshape(w * P, e // 16).astype(np.int16)


def _etile(v2d: np.ndarray) -> np.ndarray:
    """(W, E) -> (W*128, T) with [w*128+p, t] = v[w, t*128+p] (per-tile
    edge-partition layout)."""
    w, e = v2d.shape
    t = v2d.reshape(w, e // P, P).transpose(0, 2, 1)           # (W, 128, T)
    return t.reshape(w * P, e // P)


def _preprocess(inputs):
    x = np.asarray(inputs["x"], np.float32)
    ei = np.asarray(inputs["edge_index"]).astype(np.int64)
    var_idx = np.asarray(inputs["var_node_idx"]).astype(np.int64)
    wt = np.asarray(inputs["wt_onehot"], np.float32)
    mut = np.asarray(inputs["mut_onehot"], np.float32)

    src = np.concatenate([ei[0], np.arange(N, dtype=np.int64)])
    dst = np.concatenate([ei[1], np.arange(N, dtype=np.int64)])

    order = np.argsort(dst, kind="stable")
    dst_s = dst[order]

    core_of = dst_s // NLOC_REAL
    dloc = dst_s - core_of * NLOC_REAL                      # local dst 0..3749

    # balance edge counts across windows: greedily pack destinations (by
    # in-degree, heaviest first) into the 30 windows of 128 slots each.  The
    # resulting node->padded-position permutation is applied consistently to
    # xt columns, edge sources, edge destinations and var_node_idx.
    import heapq
    pos_of = np.zeros((NCORES, NLOC_REAL), np.int64)
    for c in range(NCORES):
        cnts = np.bincount(dloc[core_of == c], minlength=NLOC_REAL)
        wfill = np.zeros(WIN, np.int64)
        heap = [(0, w) for w in range(WIN)]
        heapq.heapify(heap)
        for dl in np.argsort(-cnts, kind="stable"):
            while True:
                load, w = heapq.heappop(heap)
                if wfill[w] < P:
                    break
            pos_of[c, dl] = w * P + wfill[w]
            wfill[w] += 1
            heapq.heappush(heap, (load + int(cnts[dl]), w))

    # re-derive positions under the permutation
    s_core = src // NLOC_REAL
    src_pad = s_core * NLOC + pos_of[s_core, src - s_core * NLOC_REAL]
    src_pad = src_pad[order]
    wpos = pos_of[core_of, dloc]
    win_of = wpos // P

    flat = core_of * WIN + win_of
    counts = np.bincount(flat, minlength=NCORES * WIN)
    ew = int(((counts.max() + P - 1) // P) * P)

    per_core = []
    for c in range(NCORES):
        sel = core_of == c
        sp_c, dl_c, w_c = src_pad[sel], wpos[sel], win_of[sel]
        srcw = np.zeros((WIN, ew), np.int64)
        drel = np.zeros((WIN, ew), np.float32)
        ebia = np.full((WIN, ew), PAD_BIAS, np.float32)
        for w in range(WIN):
            m = w_c == w
            k = int(m.sum())
            # order the window's edges by source for HBM locality in the xl
            # gather
            o = np.argsort(sp_c[m], kind="stable")
            srcw[w, :k] = sp_c[m][o]
            drel[w, :k] = (dl_c[m][o] % P).astype(np.float32)
            ebia[w, :k] = 0.0
        # pack per-window metadata into one u8 blob per row-block:
        # [srcidx i16 | dstrel f16 | ebias f16]
        si = _wrap16(srcw)                         # (WIN*P, ew//16) i16
        dr_ = _etile(drel).astype(np.float16)      # (WIN*P, T)
        eb_ = _etile(ebia).astype(np.float16)
        meta = np.concatenate([
            si.view(np.uint8).reshape(WIN * P, -1),
            dr_.view(np.uint8).reshape(WIN * P, -1),
            eb_.view(np.uint8).reshape(WIN * P, -1)], axis=1)
        per_core.append(dict(meta=meta, drow=drel.astype(np.float16)))

    # ---- shared weights / constants
    def pad_kT(w, m):  # (IN_DIM, m) -> (128, KCH*m) f16 chunked layout
        wp = np.zeros((KPAD, m), np.float32)
        wp[:IN_DIM] = w
        return wp.reshape(KCH, P, m).transpose(1, 0, 2).reshape(P, KCH * m).astype(np.float16)

    def two_chunk(w):  # (256, M) -> (128, 2*M) f16
        m = w.shape[1]
        return w.reshape(2, P, m).transpose(1, 0, 2).reshape(P, 2 * m).astype(np.float16)

    att1 = np.asarray(inputs["att1"], np.float32)           # (4, 64)
    attb1 = np.broadcast_to(att1.reshape(1, HID), (P, HID)).copy()
    attb2 = np.broadcast_to(np.asarray(inputs["att2"], np.float32).reshape(1, HID),
                            (P, HID)).copy()

    def rep_bias(b):  # (HID,) -> (128, HID) f32
        return np.broadcast_to(np.asarray(b, np.float32)[None, :], (P, HID)).copy()

    hW1 = np.asarray(inputs["hW1"], np.float32)             # (296, 128)
    wlr1 = np.concatenate([np.asarray(inputs["Wl1"], np.float32),
                           np.asarray(inputs["Wr1"], np.float32)], axis=1)
    wlr2 = np.concatenate([np.asarray(inputs["Wl2"], np.float32),
                           np.asarray(inputs["Wr2"], np.float32)], axis=1)
    shared = dict(
        wlr1=pad_kT(wlr1, 2 * HID),
        wlr2=two_chunk(wlr2),
        attb1=attb1.astype(np.float16),
        attb2=attb2.astype(np.float16),
        blr1=np.concatenate([rep_bias(inputs["bl1"]), rep_bias(inputs["br1"])], 1),
        bias1=rep_bias(inputs["bias1"]),
        blr2=np.concatenate([rep_bias(inputs["bl2"]), rep_bias(inputs["br2"])], 1),
        bias2=rep_bias(inputs["bias2"]),
        hw1a=hW1[0:128].astype(np.float16),
        hw1b=hW1[128:256].astype(np.float16),
        hw1c=np.vstack([hW1[256:296], np.zeros((8, 128), np.float32)]).astype(np.float16),
        hw2=np.asarray(inputs["hW2"], np.float32).astype(np.float16),   # (128, 64)
        hw3=np.asarray(inputs["hW3"], np.float32).astype(np.float16),   # (64, 1)
        hb1=np.asarray(inputs["hb1"], np.float32).reshape(P, 1),
        hb2=np.asarray(inputs["hb2"], np.float32).reshape(64, 1),
        hb3=np.asarray(inputs["hb3"], np.float32).reshape(1, 1),
        iota=np.broadcast_to(np.arange(P, dtype=np.float16)[None, :], (P, P)).copy(),
        iotat=np.arange(P, dtype=np.float32).reshape(P, 1).copy(),
        ident=np.eye(P, dtype=np.float16),
        ones1=np.ones((1, P), np.float16),
    )

    # ---- per-core x slices, transposed + padded, chunked layout (128, KCH*NLOC)
    for c in range(NCORES):
        xp = np.zeros((KPAD, NLOC), np.float32)
        xp[:IN_DIM, pos_of[c]] = x[c * NLOC_REAL:(c + 1) * NLOC_REAL].T
        per_core[c]["xt"] = xp.reshape(KCH, P, NLOC).transpose(1, 0, 2).reshape(
            P, KCH * NLOC).astype(np.float16)

    # ---- MLP batch assignment: rows go to the core owning their var node
    vcore = var_idx // NLOC_REAL
    vloc = var_idx - vcore * NLOC_REAL
    batch_rows = []
    for c in range(NCORES):
        rows = np.nonzero(vcore == c)[0]
        assert len(rows) <= BLOC, f"core {c} has {len(rows)} batch rows > {BLOC}"
        batch_rows.append(rows)
        vi = np.zeros((1, BLOC), np.int64)
        vi[0, :len(rows)] = pos_of[c, vloc[rows]]
        per_core[c]["varloc"] = _wrap16(vi)
        wm = np.zeros((40, BLOC), np.float32)
        wm[:20, :len(rows)] = wt[rows].T
        wm[20:, :len(rows)] = mut[rows].T
        per_core[c]["wtmut"] = wm.astype(np.float16)

    return per_core, shared, batch_rows, ew


def _build(ew):
    T = ew // P
    nc = bacc.Bacc("TRN2", target_bir_lowering=False, debug=False,
                   num_devices=NCORES, num_swdge_queues=1)

    # ---------- I/O ----------
    mb = 2 * (ew // 16) + 2 * T + 2 * T      # meta bytes per partition row
    io = {}
    io["xt"] = nc.dram_tensor("xt", [P, KCH * NLOC], f16, kind="ExternalInput")
    for nm, sh, dt in (
        ("wlr1", [P, KCH * 2 * HID], f16), ("wlr2", [P, 4 * HID], f16),
        ("attb1", [P, HID], f16), ("attb2", [P, HID], f16),
        ("blr1", [P, 2 * HID], f32), ("bias1", [P, HID], f32),
        ("blr2", [P, 2 * HID], f32), ("bias2", [P, HID], f32),
        ("hw1a", [P, P], f16), ("hw1b", [P, P], f16), ("hw1c", [48, P], f16),
        ("hw2", [P, 64], f16), ("hw3", [64, 1], f16),
        ("hb1", [P, 1], f32), ("hb2", [64, 1], f32), ("hb3", [1, 1], f32),
        ("iota", [P, P], f16), ("iotat", [P, 1], f32),
        ("ident", [P, P], f16), ("ones1", [1, P], f16),
        ("meta", [WIN * P, mb], mybir.dt.uint8),
        ("drow", [WIN, ew], f16),
        ("varloc", [P, BLOC // 16], i16), ("wtmut", [40, BLOC], f16),
    ):
        io[nm] = nc.dram_tensor(nm, sh, dt, kind="ExternalInput")
    out = nc.dram_tensor("out", [1, BLOC], f32, kind="ExternalOutput")

    with tile.TileContext(nc) as tc:
        with (
            tc.tile_pool(name="const", bufs=1) as cp,
            tc.tile_pool(name="dram", bufs=1, space="DRAM") as dr,
        ):
            # resident constants
            c_ = {}
            for nm in ("wlr2", "attb1", "attb2", "bias1", "blr2", "bias2",
                       "hw1a", "hw1b", "hw1c", "hw2", "hw3", "hb1", "hb2",
                       "hb3", "iota", "iotat", "ident", "ones1",
                       "varloc", "wtmut"):
                h = io[nm]
                c_[nm] = cp.tile(list(h.shape), h.dtype, tag=nm, name=f"c_{nm}")
                nc.sync.dma_start(c_[nm][:], h[:])

            # DRAM scratch
            xl1_loc = dr.tile([NLOC, HID], f16)
            xl1_all = dr.tile([NALL, HID], f16, addr_space="Shared")
            h1_loc = dr.tile([NLOC, HID], f16)
            xl2_loc = dr.tile([NLOC, HID], f16)
            xl2_all = dr.tile([NALL, HID], f16, addr_space="Shared")
            h2_loc = dr.tile([NLOC, HID], f16)

            # ================= layer 1 =================
            with tc.tile_pool(name="l1_xr", bufs=1) as xrp:
                xr1 = xrp.tile([P, WIN, HID], f16)
                # ---------- phase A layer 1 ----------
                with (
                    tc.tile_pool(name="pa_sb", bufs=2) as sb,
                    tc.tile_pool(name="pa_xt", bufs=1) as xp,
                    tc.tile_pool(name="pa_ps", bufs=4, space="PSUM") as ps,
                ):
                    xt = xp.tile([P, KCH, NLOC], f16)
                    nc.sync.dma_start(xt[:], io["xt"][:].rearrange("p (k n) -> p k n", k=KCH))
                    wlr1 = xp.tile([P, KCH, 2 * HID], f16)
                    nc.sync.dma_start(wlr1[:], io["wlr1"][:].rearrange("p (k n) -> p k n", k=KCH))
                    blr1 = xp.tile([P, 2 * HID], f32)
                    nc.sync.dma_start(blr1[:], io["blr1"][:])
                    for nt in range(WIN):
                        pa = ps.tile([P, 2 * HID], f32, tag="pa")
                        for k in range(KCH):
                            nc.tensor.matmul(pa[:], lhsT=xt[:, k, nt * P:(nt + 1) * P],
                                             rhs=wlr1[:, k, :],
                                             start=(k == 0), stop=(k == KCH - 1))
                        o = sb.tile([P, HID], f16, tag="pao")
                        nc.vector.tensor_tensor(out=o[:], in0=pa[:, 0:HID],
                                                in1=blr1[:, 0:HID], op=OP.add)
                        nc.vector.tensor_tensor(out=xr1[:, nt, :], in0=pa[:, HID:2 * HID],
                                                in1=blr1[:, HID:2 * HID], op=OP.add)
                        nc.scalar.dma_start(xl1_loc[nt * P:(nt + 1) * P, :], o[:])

                nc.gpsimd.collective_compute(
                    "AllGather", OP.bypass, replica_groups=[list(range(NCORES))],
                    ins=[xl1_loc[:].opt()], outs=[xl1_all[:].opt()])

                _emit_mp(nc, tc, ew=ew, heads=HEADS1, xl_all=xl1_all,
                         xr_sb=xr1, h_out=h1_loc, attb=c_["attb1"],
                         bias_mat=c_["bias1"], io=io, c_=c_, tag="l1")

            # ================= layer 2 =================
            with tc.tile_pool(name="l2_xr", bufs=1) as xrp:
                xr2 = xrp.tile([P, WIN, HID], f16)
                with (
                    tc.tile_pool(name="pb_sb", bufs=2) as sb,
                    tc.tile_pool(name="pb_ht", bufs=1) as hp,
                    tc.tile_pool(name="pb_ps", bufs=4, space="PSUM") as ps,
                ):
                    ht = hp.tile([P, 2, NLOC], f16)
                    for k in range(2):
                        nc.sync.dma_start_transpose(ht[:, k, :],
                                                    h1_loc[:, k * P:(k + 1) * P])
                    blr2 = c_["blr2"]
                    for nt in range(WIN):
                        pa = ps.tile([P, 2 * HID], f32, tag="pb")
                        for k in range(2):
                            nc.tensor.matmul(
                                pa[:], lhsT=ht[:, k, nt * P:(nt + 1) * P],
                                rhs=c_["wlr2"][:, k * 2 * HID:(k + 1) * 2 * HID],
                                start=(k == 0), stop=(k == 1))
                        o = sb.tile([P, HID], f16, tag="pbo")
                        nc.vector.tensor_tensor(out=o[:], in0=pa[:, 0:HID],
                                                in1=blr2[:, 0:HID], op=OP.add)
                        nc.vector.tensor_tensor(out=xr2[:, nt, :], in0=pa[:, HID:2 * HID],
                                                in1=blr2[:, HID:2 * HID], op=OP.add)
                        nc.scalar.dma_start(xl2_loc[nt * P:(nt + 1) * P, :], o[:])

                nc.gpsimd.collective_compute(
                    "AllGather", OP.bypass, replica_groups=[list(range(NCORES))],
                    ins=[xl2_loc[:].opt()], outs=[xl2_all[:].opt()])

                _emit_mp(nc, tc, ew=ew, heads=1, xl_all=xl2_all,
                         xr_sb=xr2, h_out=h2_loc, attb=c_["attb2"],
                         bias_mat=c_["bias2"], io=io, c_=c_, tag="l2")

            # ---------- MLP head ----------
            with (
                tc.tile_pool(name="mlp_sb", bufs=2) as sb,
                tc.tile_pool(name="mlp_ps", bufs=2, space="PSUM") as ps,
            ):
                sel = sb.tile([P, 2, BLOC], f16)
                nc.gpsimd.dma_gather(sel[:], h2_loc[:], c_["varloc"][:],
                                     num_idxs=BLOC, num_idxs_reg=BLOC,
                                     elem_size=HID, transpose=True)
                for c0, cn in ((0, 512), (512, BLOC - 512)):
                    z1p = ps.tile([P, 512], f32, tag="z1p")
                    nc.tensor.matmul(z1p[:, :cn], lhsT=c_["hw1a"][:],
                                     rhs=sel[:, 0, c0:c0 + cn], start=True, stop=False)
                    nc.tensor.matmul(z1p[:, :cn], lhsT=c_["hw1b"][:],
                                     rhs=sel[:, 1, c0:c0 + cn], start=False, stop=False)
                    nc.tensor.matmul(z1p[:, :cn], lhsT=c_["hw1c"][0:40, :],
                                     rhs=c_["wtmut"][:, c0:c0 + cn], start=False, stop=True)
                    z1 = sb.tile([P, 512], f16, tag="z1")
                    nc.scalar.activation(z1[:, :cn], z1p[:, :cn], AF.Relu,
                                         bias=c_["hb1"][:])
                    z2p = ps.tile([64, 512], f32, tag="z2p")
                    nc.tensor.matmul(z2p[:, :cn], lhsT=c_["hw2"][:],
                                     rhs=z1[:, :cn], start=True, stop=True)
                    z2 = sb.tile([64, 512], f16, tag="z2")
                    nc.scalar.activation(z2[:, :cn], z2p[:, :cn], AF.Relu,
                                         bias=c_["hb2"][:])
                    z3p = ps.tile([1, 512], f32, tag="z3p")
                    nc.tensor.matmul(z3p[:, :cn], lhsT=c_["hw3"][:],
                                     rhs=z2[:, :cn], start=True, stop=True)
                    z3 = sb.tile([1, 512], f32, tag="z3")
                    nc.scalar.activation(z3[:, :cn], z3p[:, :cn], AF.Identity,
                                         bias=c_["hb3"][:])
                    nc.sync.dma_start(out[0:1, c0:c0 + cn], z3[:, :cn])

    nc.compile()
    return nc


def _emit_mp(nc, tc, *, ew, heads, xl_all, xr_sb, h_out, attb, bias_mat,
             io, c_, tag):
    """Message passing for one GATv2 layer.

    Software-pipelined over windows so each engine's FIFO never stalls on a
    cross-engine dependency of the same window:
      stage A (window w):   meta/drow loads, xl gathers, S and S^T builds
      stage B (window w-1): v matmuls, prelu, logits, exp, gw, agg/den
      stage C (window w-2): normalize + bias + ELU + store
    """
    T = ew // P
    CW = HID // heads
    i16b = 2 * (ew // 16)
    mb = i16b + 2 * T + 2 * T
    chunks = []
    o = 0
    while o < ew:
        n = min(896, ew - o)
        chunks.append((o, n))
        o += n
    regs = {n: nc.gpsimd.to_reg(n) for _, n in set(chunks)}
    tiles = {}
    with (
        tc.tile_pool(name=f"{tag}_g", bufs=2) as gp,
        tc.tile_pool(name=f"{tag}_sb", bufs=2) as sb,
        tc.tile_pool(name=f"{tag}_sm", bufs=2) as sm,
        tc.tile_pool(name=f"{tag}_pm", bufs=2, space="PSUM") as pmp,
        tc.tile_pool(name=f"{tag}_pa", bufs=2, space="PSUM") as pap,
    ):
        # zero the gather buffers once: rows skipped by negative idxs must
        # not contain NaN bit patterns (they are multiplied by exp()==0)
        for _ in range(4):
            gz = gp.tile([P, T, HID], f16, tag="g", bufs=4)
            nc.vector.memset(gz[:], 0.0)
        # att row replicated T times so the logit multiply has unit strides
        attbt = sb.tile([P, T, HID], f16, tag="attbt", bufs=1)
        nc.vector.tensor_tensor(
            out=attbt[:],
            in0=attb[:].rearrange("p (o c) -> p o c", o=1).to_broadcast([P, T, HID]),
            in1=attb[:].rearrange("p (o c) -> p o c", o=1).to_broadcast([P, T, HID]),
            op=OP.max)
        for it in range(WIN + 2):
            # ---------------- stage C: window it-2 ----------------
            if it >= 2:
                w = it - 2
                t_ = tiles.pop(w)
                ad, dn = t_["ad"], t_["dn"]
                den = sb.tile([P, heads], f32, tag="den")
                nc.vector.tensor_scalar_add(den[:], dn[:], 1e-16)
                rden = sb.tile([P, heads], f32, tag="rden")
                nc.vector.reciprocal(rden[:], den[:])
                hn = sb.tile([P, HID], f32, tag="hn")
                nc.vector.tensor_tensor(
                    out=hn[:].rearrange("p (h c) -> p h c", h=heads),
                    in0=ad[:, 0:HID].rearrange("p (h c) -> p h c", h=heads),
                    in1=rden[:].rearrange("p (h o) -> p h o", o=1).to_broadcast([P, heads, CW]),
                    op=OP.mult)
                hb = sb.tile([P, HID], f32, tag="hb")
                nc.vector.tensor_tensor(out=hb[:], in0=hn[:], in1=bias_mat[:], op=OP.add)
                # ELU(x) = relu(x) + exp(min(x, 0)) - 1
                mn = sb.tile([P, HID], f32, tag="mn")
                nc.vector.scalar_tensor_tensor(out=mn[:], in0=hb[:], scalar=0.0,
                                               in1=hb[:], op0=OP.min, op1=OP.bypass)
                exe = sb.tile([P, HID], f32, tag="exe")
                nc.scalar.activation(exe[:], mn[:], AF.Exp)
                el = sb.tile([P, HID], f32, tag="el")
                nc.vector.scalar_tensor_tensor(out=el[:], in0=hb[:], scalar=0.0,
                                               in1=exe[:], op0=OP.max, op1=OP.add)
                h_t = sb.tile([P, HID], f16, tag="h_t")
                nc.vector.scalar_tensor_tensor(out=h_t[:], in0=el[:], scalar=-1.0,
                                               in1=el[:], op0=OP.add, op1=OP.bypass)
                nc.scalar.dma_start(h_out[w * P:(w + 1) * P, :], h_t[:])

            # ---------------- stage A: window it ----------------
            if it < WIN:
                w = it
                meta = sm.tile([P, mb], mybir.dt.uint8, tag="meta")
                nc.sync.dma_start(meta[:], io["meta"][w * P:(w + 1) * P, :])
                si = meta[:, 0:i16b].bitcast(i16)
                drl = meta[:, i16b:i16b + 2 * T].bitcast(f16)
                eb = meta[:, i16b + 2 * T:i16b + 4 * T].bitcast(f16)

                g = gp.tile([P, T, HID], f16, tag="g", bufs=4)
                for o, n in chunks:
                    nc.gpsimd.dma_gather(g[:, o // P:(o + n) // P, :], xl_all[:],
                                         si[:, o // 16:(o + n) // 16],
                                         num_idxs=n, num_idxs_reg=regs[n],
                                         elem_size=HID, transpose=False)

                S = sb.tile([P, T, P], f16, tag="S", bufs=3)
                nc.vector.tensor_tensor(
                    out=S[:],
                    in0=drl[:].rearrange("p (t o) -> p t o", o=1).to_broadcast([P, T, P]),
                    in1=c_["iota"][:].rearrange("p (o d) -> p o d", o=1).to_broadcast([P, T, P]),
                    op=OP.is_equal)

                sT = sb.tile([P, T, P], f16, tag="sT")
                db = sb.tile([P, ew], f16, tag="db", bufs=1)
                nc.sync.dma_start(db[:], io["drow"][w:w + 1, :].to_broadcast([P, ew]))
                nc.vector.tensor_scalar(
                    sT[:].rearrange("p t e -> p (t e)"), db[:],
                    c_["iotat"][:], None, OP.is_equal)
                tiles[w] = dict(g=g, S=S, sT=sT, eb=eb)

            # ---------------- stage B: window it-1 ----------------
            # processed in two half-window slices so the cross-engine chain
            # (v -> prelu -> logits -> exp -> gw -> agg) pipelines at half-
            # window granularity
            if 1 <= it <= WIN:
                w = it - 1
                t_ = tiles[w]
                g, S, sT, eb = t_["g"], t_["S"], t_["sT"], t_["eb"]

                lr = sb.tile([P, T, HID], f16, tag="lr", bufs=1)
                la = sb.tile([P, T, HID], f16, tag="la", bufs=1)
                lg = sb.tile([P, T, heads], f16, tag="lg")
                ex = sb.tile([P, T, heads], f16, tag="ex")
                ad = pap.tile([P, 512], f32, tag="ad")
                dn = pap.tile([P, heads], f32, tag="dn")
                TH = (T + 1) // 2
                for h0 in range(0, T, TH):
                    h1 = min(h0 + TH, T)
                    for b0 in range(h0, h1, BATCH_M):
                        b1 = min(b0 + BATCH_M, h1)
                        pm = pmp.tile([P, BATCH_M, HID], f32, tag="pm")
                        # keep each slot's 2-matmul group consecutive:
                        # start=True wipes the whole bank's has_written bits
                        for t in range(b0, b1):
                            nc.tensor.matmul(pm[:, t - b0, :], lhsT=sT[:, t, :],
                                             rhs=xr_sb[:, w, :], start=True, stop=False)
                            nc.tensor.matmul(pm[:, t - b0, :], lhsT=c_["ident"][:],
                                             rhs=g[:, t, :], start=False, stop=True)
                        nc.scalar.activation(lr[:, b0:b1, :], pm[:, 0:b1 - b0, :],
                                             AF.Prelu, alpha=NEG)

                    nc.vector.tensor_tensor(
                        out=la[:, h0:h1, :], in0=lr[:, h0:h1, :],
                        in1=attbt[:, h0:h1, :], op=OP.mult)
                    with nc.allow_low_precision("f16 logit reduce; verified 5.9e-3"):
                        nc.vector.tensor_reduce(
                            out=lg[:, h0:h1, :].rearrange("p t h -> p (t h)"),
                            in_=la[:, h0:h1, :].rearrange("p t (h c) -> p (t h) c", h=heads),
                            axis=mybir.AxisListType.X, op=OP.add)
                    nc.vector.tensor_tensor(
                        out=lg[:, h0:h1, :], in0=lg[:, h0:h1, :],
                        in1=eb[:, h0:h1].rearrange("p (t o) -> p t o", o=1).to_broadcast(
                            [P, h1 - h0, heads]),
                        op=OP.add)
                    nc.scalar.activation(ex[:, h0:h1, :], lg[:, h0:h1, :], AF.Exp)

                    nc.vector.tensor_tensor(
                        out=g[:, h0:h1, :].rearrange("p t (h c) -> p t h c", h=heads),
                        in0=g[:, h0:h1, :].rearrange("p t (h c) -> p t h c", h=heads),
                        in1=ex[:, h0:h1, :].rearrange("p t (h o) -> p t h o", o=1).to_broadcast(
                            [P, h1 - h0, heads, CW]),
                        op=OP.mult)
                    # agg and den keep their groups open across all T tiles;
                    # they live in different banks (start=True wipes bank bits)
                    for t in range(h0, h1):
                        nc.tensor.matmul(ad[:, 0:HID], lhsT=S[:, t, :], rhs=g[:, t, :],
                                         start=(t == 0), stop=(t == T - 1))
                        nc.tensor.matmul(dn[:], lhsT=S[:, t, :],
                                         rhs=ex[:, t, :], start=(t == 0), stop=(t == T - 1))
                t_["ad"], t_["dn"] = ad, dn


def kernel(**inputs):
    per_core, shared, batch_rows, ew = _preprocess(inputs)

    if ew not in _nc_cache:
        _nc_cache[ew] = _build(ew)
    nc = _nc_cache[ew]

    in_maps = []
    for c in range(NCORES):
        m = dict(shared)
        m.update(per_core[c])
        in_maps.append({k: np.ascontiguousarray(v) for k, v in m.items()})

    res = run_bass_kernel_spmd(nc, in_maps, core_ids=list(range(NCORES)))

    B = len(np.asarray(inputs["var_node_idx"]))
    out = np.zeros((B,), np.float32)
    for c in range(NCORES):
        rows = batch_rows[c]
        out[rows] = res.results[c]["out"][0, :len(rows)]
    return out


# revision 16
# speedup vs baseline: 1.1014x; 1.1014x over previous
"""Trainium2 Bass kernel for the 2-layer GATv2 + MLP-head model (nn_GAT_21028159881586).

Strategy (8 NeuronCores, SPMD single NEFF):
  * Destination-block partitioning: global nodes are split into 8 slices of
    3750 (padded to 3840 = 30 windows x 128 per core).  Core c owns all edges
    whose destination lands in its slice, so segment softmax + aggregation are
    core-local.
  * Per layer: data-parallel node transforms xl = x@Wl+bl / xr = x@Wr+br on
    the local slice; xr stays resident in SBUF, xl is AllGathered across the
    8 cores, then 30 windows of 128 destinations each are processed.
  * Per window (the V2 pipeline -- exactly ONE row-major gather stream):
      - dma_gather of xl rows (by edge source) in (edge, channel) layout only
        (3 chunks of 896 idxs).  The (channel, edge) copy and the xr[dst]
        gather of V1 are gone: xr[dst] is reconstructed on the PE from the
        128 window xr rows via the transposed 0/1 scatter matrix, and the
        logits contract over channels on the DVE instead of the PE.
      - S   [e,d] = (drl[e] == d)   built by one DVE compare  (agg lhsT)
        S^T [d,e]                   built by DVE compare against a partition-
        broadcast of drl (broadcast done by a stride-0 DMA or a K=1 matmul)
      - per tile t: PSUM m = S^T.T @ xr_window + I.T @ xl_src  (PE), then
        ACT Prelu(m) -> lr, DVE lr *= att (broadcast), DVE segmented reduce
        -> logits, += pad bias, ACT exp, DVE xl_src *= exp (in place), PE
        aggregation matmuls into PSUM [agg | den].
      - normalize by 1/den, add bias, ELU, write the 128 output rows.
  * Softmax max-subtraction is skipped (logits are O(1); exp cannot overflow).
  * MLP head: batch rows are assigned to the core owning their var node, the
    selected h2 rows are dma_gathered transposed, and the 3-layer MLP runs
    fully transposed.

Everything runs in fp16 with fp32 PSUM accumulation.
"""

import numpy as np

import concourse.bacc as bacc
import concourse.tile as tile
import concourse.mybir as mybir
from concourse.bass_utils import run_bass_kernel_spmd

P = 128
NCORES = 8
N = 30000
NLOC_REAL = 3750          # real nodes per core
WIN = 30                  # destination windows per core
NLOC = WIN * P            # 3840 padded nodes per core
NALL = NCORES * NLOC      # 30720 padded global nodes
IN_DIM = 1281
KCH = 11                  # input-dim chunks of 128
KPAD = KCH * P            # 1408
HID = 256
HEADS1 = 4
BLOC = 640                # padded batch rows per core (actual max ~554)
NEG = 0.2
PAD_BIAS = -30000.0
BATCH_M = 4               # tiles per PSUM m batch

f32 = mybir.dt.float32
f16 = mybir.dt.float16
i16 = mybir.dt.int16
AF = mybir.ActivationFunctionType
OP = mybir.AluOpType

USE_BCAST_DMA = True      # stride-0 partition broadcast via DMA for drl row

_nc_cache = {}


def _wrap16(idx2d: np.ndarray) -> np.ndarray:
    """(W, E) int -> (W*128, E//16) int16, wrapped in 16 partitions, replicated
    across the 8 gpsimd cores."""
    w, e = idx2d.shape
    assert e % 16 == 0
    t = idx2d.reshape(w, e // 16, 16).transpose(0, 2, 1)       # (W, 16, E/16)
    return np.tile(t, (1, 8, 1)).reshape(w * P, e // 16).astype(np.int16)


def _etile(v2d: np.ndarray) -> np.ndarray:
    """(W, E) -> (W*128, T) with [w*128+p, t] = v[w, t*128+p] (per-tile
    edge-partition layout)."""
    w, e = v2d.shape
    t = v2d.reshape(w, e // P, P).transpose(0, 2, 1)           # (W, 128, T)
    return t.reshape(w * P, e // P)


def _preprocess(inputs):
    x = np.asarray(inputs["x"], np.float32)
    ei = np.asarray(inputs["edge_index"]).astype(np.int64)
    var_idx = np.asarray(inputs["var_node_idx"]).astype(np.int64)
    wt = np.asarray(inputs["wt_onehot"], np.float32)
    mut = np.asarray(inputs["mut_onehot"], np.float32)

    src = np.concatenate([ei[0], np.arange(N, dtype=np.int64)])
    dst = np.concatenate([ei[1], np.arange(N, dtype=np.int64)])

    order = np.argsort(dst, kind="stable")
    dst_s = dst[order]

    core_of = dst_s // NLOC_REAL
    dloc = dst_s - core_of * NLOC_REAL                      # local dst 0..3749

    # balance edge counts across windows: greedily pack destinations (by
    # in-degree, heaviest first) into the 30 windows of 128 slots each.  The
    # resulting node->padded-position permutation is applied consistently to
    # xt columns, edge sources, edge destinations and var_node_idx.
    import heapq
    pos_of = np.zeros((NCORES, NLOC_REAL), np.int64)
    for c in range(NCORES):
        cnts = np.bincount(dloc[core_of == c], minlength=NLOC_REAL)
        wfill = np.zeros(WIN, np.int64)
        heap = [(0, w) for w in range(WIN)]
        heapq.heapify(heap)
        for dl in np.argsort(-cnts, kind="stable"):
            while True:
                load, w = heapq.heappop(heap)
                if wfill[w] < P:
                    break
            pos_of[c, dl] = w * P + wfill[w]
            wfill[w] += 1
            heapq.heappush(heap, (load + int(cnts[dl]), w))

    # re-derive positions under the permutation
    s_core = src // NLOC_REAL
    src_pad = s_core * NLOC + pos_of[s_core, src - s_core * NLOC_REAL]
    src_pad = src_pad[order]
    wpos = pos_of[core_of, dloc]
    win_of = wpos // P

    flat = core_of * WIN + win_of
    counts = np.bincount(flat, minlength=NCORES * WIN)
    ew = int(((counts.max() + P - 1) // P) * P)

    per_core = []
    for c in range(NCORES):
        sel = core_of == c
        sp_c, dl_c, w_c = src_pad[sel], wpos[sel], win_of[sel]
        srcw = np.zeros((WIN, ew), np.int64)
        # padding edges carry dstrel=-1: their S / S^T columns are all-zero,
        # so they contribute to neither the aggregation nor the denominator
        drel = np.full((WIN, ew), -1.0, np.float32)
        for w in range(WIN):
            m = w_c == w
            k = int(m.sum())
            # order the window's edges by source for HBM locality in the xl
            # gather
            o = np.argsort(sp_c[m], kind="stable")
            srcw[w, :k] = sp_c[m][o]
            drel[w, :k] = (dl_c[m][o] % P).astype(np.float32)
        # pack per-window metadata into one u8 blob per row-block:
        # [srcidx i16 | dstrel f16]
        si = _wrap16(srcw)                         # (WIN*P, ew//16) i16
        dr_ = _etile(drel).astype(np.float16)      # (WIN*P, T)
        meta = np.concatenate([
            si.view(np.uint8).reshape(WIN * P, -1),
            dr_.view(np.uint8).reshape(WIN * P, -1)], axis=1)
        per_core.append(dict(meta=meta, drow=drel.astype(np.float16)))

    # ---- shared weights / constants
    def pad_kT(w, m):  # (IN_DIM, m) -> (128, KCH*m) f16 chunked layout
        wp = np.zeros((KPAD, m), np.float32)
        wp[:IN_DIM] = w
        return wp.reshape(KCH, P, m).transpose(1, 0, 2).reshape(P, KCH * m).astype(np.float16)

    def two_chunk(w):  # (256, M) -> (128, 2*M) f16
        m = w.shape[1]
        return w.reshape(2, P, m).transpose(1, 0, 2).reshape(P, 2 * m).astype(np.float16)

    att1 = np.asarray(inputs["att1"], np.float32)           # (4, 64)
    attb1 = np.broadcast_to(att1.reshape(1, HID), (P, HID)).copy()
    attb2 = np.broadcast_to(np.asarray(inputs["att2"], np.float32).reshape(1, HID),
                            (P, HID)).copy()

    def rep_bias(b):  # (HID,) -> (128, HID) f32
        return np.broadcast_to(np.asarray(b, np.float32)[None, :], (P, HID)).copy()

    hW1 = np.asarray(inputs["hW1"], np.float32)             # (296, 128)
    wlr1 = np.concatenate([np.asarray(inputs["Wl1"], np.float32),
                           np.asarray(inputs["Wr1"], np.float32)], axis=1)
    wlr2 = np.concatenate([np.asarray(inputs["Wl2"], np.float32),
                           np.asarray(inputs["Wr2"], np.float32)], axis=1)
    shared = dict(
        wlr1=pad_kT(wlr1, 2 * HID),
        wlr2=two_chunk(wlr2),
        attb1=attb1.astype(np.float16),
        attb2=attb2.astype(np.float16),
        blr1=np.concatenate([rep_bias(inputs["bl1"]), rep_bias(inputs["br1"])], 1),
        bias1=rep_bias(inputs["bias1"]),
        blr2=np.concatenate([rep_bias(inputs["bl2"]), rep_bias(inputs["br2"])], 1),
        bias2=rep_bias(inputs["bias2"]),
        hw1a=hW1[0:128].astype(np.float16),
        hw1b=hW1[128:256].astype(np.float16),
        hw1c=np.vstack([hW1[256:296], np.zeros((8, 128), np.float32)]).astype(np.float16),
        hw2=np.asarray(inputs["hW2"], np.float32).astype(np.float16),   # (128, 64)
        hw3=np.asarray(inputs["hW3"], np.float32).astype(np.float16),   # (64, 1)
        hb1=np.asarray(inputs["hb1"], np.float32).reshape(P, 1),
        hb2=np.asarray(inputs["hb2"], np.float32).reshape(64, 1),
        hb3=np.asarray(inputs["hb3"], np.float32).reshape(1, 1),
        iota=np.broadcast_to(np.arange(P, dtype=np.float16)[None, :], (P, P)).copy(),
        iotat=np.arange(P, dtype=np.float32).reshape(P, 1).copy(),
        ident=np.eye(P, dtype=np.float16),
        ones1=np.ones((1, P), np.float16),
        onesc=np.ones((P, 1), np.float16),
    )

    # ---- per-core x slices, transposed + padded, chunked layout (128, KCH*NLOC)
    for c in range(NCORES):
        xp = np.zeros((KPAD, NLOC), np.float32)
        xp[:IN_DIM, pos_of[c]] = x[c * NLOC_REAL:(c + 1) * NLOC_REAL].T
        per_core[c]["xt"] = xp.reshape(KCH, P, NLOC).transpose(1, 0, 2).reshape(
            P, KCH * NLOC).astype(np.float16)

    # ---- MLP batch assignment: rows go to the core owning their var node
    vcore = var_idx // NLOC_REAL
    vloc = var_idx - vcore * NLOC_REAL
    batch_rows = []
    for c in range(NCORES):
        rows = np.nonzero(vcore == c)[0]
        assert len(rows) <= BLOC, f"core {c} has {len(rows)} batch rows > {BLOC}"
        batch_rows.append(rows)
        vi = np.zeros((1, BLOC), np.int64)
        vi[0, :len(rows)] = pos_of[c, vloc[rows]]
        per_core[c]["varloc"] = _wrap16(vi)
        wm = np.zeros((40, BLOC), np.float32)
        wm[:20, :len(rows)] = wt[rows].T
        wm[20:, :len(rows)] = mut[rows].T
        per_core[c]["wtmut"] = wm.astype(np.float16)

    return per_core, shared, batch_rows, ew


def _build(ew):
    T = ew // P
    nc = bacc.Bacc("TRN2", target_bir_lowering=False, debug=False,
                   num_devices=NCORES, num_swdge_queues=1)

    # ---------- I/O ----------
    mb = 2 * (ew // 16) + 2 * T      # meta bytes per partition row
    io = {}
    io["xt"] = nc.dram_tensor("xt", [P, KCH * NLOC], f16, kind="ExternalInput")
    for nm, sh, dt in (
        ("wlr1", [P, KCH * 2 * HID], f16), ("wlr2", [P, 4 * HID], f16),
        ("attb1", [P, HID], f16), ("attb2", [P, HID], f16),
        ("blr1", [P, 2 * HID], f32), ("bias1", [P, HID], f32),
        ("blr2", [P, 2 * HID], f32), ("bias2", [P, HID], f32),
        ("hw1a", [P, P], f16), ("hw1b", [P, P], f16), ("hw1c", [48, P], f16),
        ("hw2", [P, 64], f16), ("hw3", [64, 1], f16),
        ("hb1", [P, 1], f32), ("hb2", [64, 1], f32), ("hb3", [1, 1], f32),
        ("iota", [P, P], f16), ("iotat", [P, 1], f32),
        ("ident", [P, P], f16), ("ones1", [1, P], f16), ("onesc", [P, 1], f16),
        ("meta", [WIN * P, mb], mybir.dt.uint8),
        ("drow", [WIN, ew], f16),
        ("varloc", [P, BLOC // 16], i16), ("wtmut", [40, BLOC], f16),
    ):
        io[nm] = nc.dram_tensor(nm, sh, dt, kind="ExternalInput")
    out = nc.dram_tensor("out", [1, BLOC], f32, kind="ExternalOutput")

    with tile.TileContext(nc) as tc:
        with (
            tc.tile_pool(name="const", bufs=1) as cp,
            tc.tile_pool(name="dram", bufs=1, space="DRAM") as dr,
        ):
            # resident constants
            c_ = {}
            for nm in ("wlr2", "attb1", "attb2", "bias1", "blr2", "bias2",
                       "hw1a", "hw1b", "hw1c", "hw2", "hw3", "hb1", "hb2",
                       "hb3", "iota", "iotat", "ident", "ones1", "onesc",
                       "varloc", "wtmut"):
                h = io[nm]
                c_[nm] = cp.tile(list(h.shape), h.dtype, tag=nm, name=f"c_{nm}")
                nc.sync.dma_start(c_[nm][:], h[:])

            # DRAM scratch
            xl1_loc = dr.tile([NLOC, HID], f16)
            xl1_all = dr.tile([NALL, HID], f16, addr_space="Shared")
            h1_loc = dr.tile([NLOC, HID], f16)
            xl2_loc = dr.tile([NLOC, HID], f16)
            xl2_all = dr.tile([NALL, HID], f16, addr_space="Shared")
            h2_loc = dr.tile([NLOC, HID], f16)

            # ================= layer 1 =================
            with tc.tile_pool(name="l1_xr", bufs=1) as xrp:
                xr1 = xrp.tile([P, WIN, HID], f16)
                # ---------- phase A layer 1 ----------
                with (
                    tc.tile_pool(name="pa_sb", bufs=2) as sb,
                    tc.tile_pool(name="pa_xt", bufs=1) as xp,
                    tc.tile_pool(name="pa_ps", bufs=4, space="PSUM") as ps,
                ):
                    xt = xp.tile([P, KCH, NLOC], f16)
                    nc.sync.dma_start(xt[:], io["xt"][:].rearrange("p (k n) -> p k n", k=KCH))
                    wlr1 = xp.tile([P, KCH, 2 * HID], f16)
                    nc.sync.dma_start(wlr1[:], io["wlr1"][:].rearrange("p (k n) -> p k n", k=KCH))
                    blr1 = xp.tile([P, 2 * HID], f32)
                    nc.sync.dma_start(blr1[:], io["blr1"][:])
                    for nt in range(WIN):
                        pa = ps.tile([P, 2 * HID], f32, tag="pa")
                        for k in range(KCH):
                            nc.tensor.matmul(pa[:], lhsT=xt[:, k, nt * P:(nt + 1) * P],
                                             rhs=wlr1[:, k, :],
                                             start=(k == 0), stop=(k == KCH - 1))
                        o = sb.tile([P, HID], f16, tag="pao")
                        nc.vector.tensor_tensor(out=o[:], in0=pa[:, 0:HID],
                                                in1=blr1[:, 0:HID], op=OP.add)
                        nc.vector.tensor_tensor(out=xr1[:, nt, :], in0=pa[:, HID:2 * HID],
                                                in1=blr1[:, HID:2 * HID], op=OP.add)
                        nc.scalar.dma_start(xl1_loc[nt * P:(nt + 1) * P, :], o[:])

                nc.gpsimd.collective_compute(
                    "AllGather", OP.bypass, replica_groups=[list(range(NCORES))],
                    ins=[xl1_loc[:].opt()], outs=[xl1_all[:].opt()])

                _emit_mp(nc, tc, ew=ew, heads=HEADS1, xl_all=xl1_all,
                         xr_sb=xr1, h_out=h1_loc, attb=c_["attb1"],
                         bias_mat=c_["bias1"], io=io, c_=c_, tag="l1")

            # ================= layer 2 =================
            with tc.tile_pool(name="l2_xr", bufs=1) as xrp:
                xr2 = xrp.tile([P, WIN, HID], f16)
                with (
                    tc.tile_pool(name="pb_sb", bufs=2) as sb,
                    tc.tile_pool(name="pb_ht", bufs=1) as hp,
                    tc.tile_pool(name="pb_ps", bufs=4, space="PSUM") as ps,
                ):
                    ht = hp.tile([P, 2, NLOC], f16)
                    for k in range(2):
                        nc.sync.dma_start_transpose(ht[:, k, :],
                                                    h1_loc[:, k * P:(k + 1) * P])
                    blr2 = c_["blr2"]
                    for nt in range(WIN):
                        pa = ps.tile([P, 2 * HID], f32, tag="pb")
                        for k in range(2):
                            nc.tensor.matmul(
                                pa[:], lhsT=ht[:, k, nt * P:(nt + 1) * P],
                                rhs=c_["wlr2"][:, k * 2 * HID:(k + 1) * 2 * HID],
                                start=(k == 0), stop=(k == 1))
                        o = sb.tile([P, HID], f16, tag="pbo")
                        nc.vector.tensor_tensor(out=o[:], in0=pa[:, 0:HID],
                                                in1=blr2[:, 0:HID], op=OP.add)
                        nc.vector.tensor_tensor(out=xr2[:, nt, :], in0=pa[:, HID:2 * HID],
                                                in1=blr2[:, HID:2 * HID], op=OP.add)
                        nc.scalar.dma_start(xl2_loc[nt * P:(nt + 1) * P, :], o[:])

                nc.gpsimd.collective_compute(
                    "AllGather", OP.bypass, replica_groups=[list(range(NCORES))],
                    ins=[xl2_loc[:].opt()], outs=[xl2_all[:].opt()])

                _emit_mp(nc, tc, ew=ew, heads=1, xl_all=xl2_all,
                         xr_sb=xr2, h_out=h2_loc, attb=c_["attb2"],
                         bias_mat=c_["bias2"], io=io, c_=c_, tag="l2")

            # ---------- MLP head ----------
            with (
                tc.tile_pool(name="mlp_sb", bufs=2) as sb,
                tc.tile_pool(name="mlp_ps", bufs=2, space="PSUM") as ps,
            ):
                sel = sb.tile([P, 2, BLOC], f16)
                nc.gpsimd.dma_gather(sel[:], h2_loc[:], c_["varloc"][:],
                                     num_idxs=BLOC, num_idxs_reg=BLOC,
                                     elem_size=HID, transpose=True)
                for c0, cn in ((0, 512), (512, BLOC - 512)):
                    z1p = ps.tile([P, 512], f32, tag="z1p")
                    nc.tensor.matmul(z1p[:, :cn], lhsT=c_["hw1a"][:],
                                     rhs=sel[:, 0, c0:c0 + cn], start=True, stop=False)
                    nc.tensor.matmul(z1p[:, :cn], lhsT=c_["hw1b"][:],
                                     rhs=sel[:, 1, c0:c0 + cn], start=False, stop=False)
                    nc.tensor.matmul(z1p[:, :cn], lhsT=c_["hw1c"][0:40, :],
                                     rhs=c_["wtmut"][:, c0:c0 + cn], start=False, stop=True)
                    z1 = sb.tile([P, 512], f16, tag="z1")
                    nc.scalar.activation(z1[:, :cn], z1p[:, :cn], AF.Relu,
                                         bias=c_["hb1"][:])
                    z2p = ps.tile([64, 512], f32, tag="z2p")
                    nc.tensor.matmul(z2p[:, :cn], lhsT=c_["hw2"][:],
                                     rhs=z1[:, :cn], start=True, stop=True)
                    z2 = sb.tile([64, 512], f16, tag="z2")
                    nc.scalar.activation(z2[:, :cn], z2p[:, :cn], AF.Relu,
                                         bias=c_["hb2"][:])
                    z3p = ps.tile([1, 512], f32, tag="z3p")
                    nc.tensor.matmul(z3p[:, :cn], lhsT=c_["hw3"][:],
                                     rhs=z2[:, :cn], start=True, stop=True)
                    z3 = sb.tile([1, 512], f32, tag="z3")
                    nc.scalar.activation(z3[:, :cn], z3p[:, :cn], AF.Identity,
                                         bias=c_["hb3"][:])
                    nc.sync.dma_start(out[0:1, c0:c0 + cn], z3[:, :cn])

    nc.compile()
    return nc


def _emit_mp(nc, tc, *, ew, heads, xl_all, xr_sb, h_out, attb, bias_mat,
             io, c_, tag):
    """Message passing for one GATv2 layer.

    Software-pipelined over windows so each engine's FIFO never stalls on a
    cross-engine dependency of the same window:
      stage A (window w):   meta/drow loads, xl gathers, S and S^T builds
      stage B (window w-1): v matmuls, prelu, logits, exp, gw, agg/den
      stage C (window w-2): normalize + bias + ELU + store
    """
    T = ew // P
    CW = HID // heads
    i16b = 2 * (ew // 16)
    mb = i16b + 2 * T
    chunks = []
    o = 0
    while o < ew:
        n = min(896, ew - o)
        chunks.append((o, n))
        o += n
    regs = {n: nc.gpsimd.to_reg(n) for _, n in set(chunks)}
    tiles = {}
    with (
        tc.tile_pool(name=f"{tag}_g", bufs=2) as gp,
        tc.tile_pool(name=f"{tag}_sb", bufs=2) as sb,
        tc.tile_pool(name=f"{tag}_sm", bufs=2) as sm,
        tc.tile_pool(name=f"{tag}_pm", bufs=2, space="PSUM") as pmp,
        tc.tile_pool(name=f"{tag}_pa", bufs=2, space="PSUM") as pap,
    ):
        # zero the gather buffers once: rows skipped by negative idxs must
        # not contain NaN bit patterns (they are multiplied by exp()==0)
        for _ in range(4):
            gz = gp.tile([P, T, HID], f16, tag="g", bufs=4)
            nc.vector.memset(gz[:], 0.0)
        # att row replicated T times so the logit multiply has unit strides
        attbt = sb.tile([P, T, HID], f16, tag="attbt", bufs=1)
        nc.vector.tensor_tensor(
            out=attbt[:],
            in0=attb[:].rearrange("p (o c) -> p o c", o=1).to_broadcast([P, T, HID]),
            in1=attb[:].rearrange("p (o c) -> p o c", o=1).to_broadcast([P, T, HID]),
            op=OP.max)
        for it in range(WIN + 2):
            # ---------------- stage C: window it-2 ----------------
            if it >= 2:
                w = it - 2
                t_ = tiles.pop(w)
                ad, dn = t_["ad"], t_["dn"]
                den = sb.tile([P, heads], f32, tag="den")
                nc.vector.tensor_scalar_add(den[:], dn[:], 1e-16)
                rden = sb.tile([P, heads], f32, tag="rden")
                nc.vector.reciprocal(rden[:], den[:])
                hn = sb.tile([P, HID], f32, tag="hn")
                nc.vector.tensor_tensor(
                    out=hn[:].rearrange("p (h c) -> p h c", h=heads),
                    in0=ad[:, 0:HID].rearrange("p (h c) -> p h c", h=heads),
                    in1=rden[:].rearrange("p (h o) -> p h o", o=1).to_broadcast([P, heads, CW]),
                    op=OP.mult)
                hb = sb.tile([P, HID], f32, tag="hb")
                nc.vector.tensor_tensor(out=hb[:], in0=hn[:], in1=bias_mat[:], op=OP.add)
                # ELU(x) = relu(x) + exp(min(x, 0)) - 1
                mn = sb.tile([P, HID], f32, tag="mn")
                nc.vector.scalar_tensor_tensor(out=mn[:], in0=hb[:], scalar=0.0,
                                               in1=hb[:], op0=OP.min, op1=OP.bypass)
                exe = sb.tile([P, HID], f32, tag="exe")
                nc.scalar.activation(exe[:], mn[:], AF.Exp)
                el = sb.tile([P, HID], f32, tag="el")
                nc.vector.scalar_tensor_tensor(out=el[:], in0=hb[:], scalar=0.0,
                                               in1=exe[:], op0=OP.max, op1=OP.add)
                h_t = sb.tile([P, HID], f16, tag="h_t")
                nc.vector.scalar_tensor_tensor(out=h_t[:], in0=el[:], scalar=-1.0,
                                               in1=el[:], op0=OP.add, op1=OP.bypass)
                nc.scalar.dma_start(h_out[w * P:(w + 1) * P, :], h_t[:])

            # ---------------- stage A: window it ----------------
            if it < WIN:
                w = it
                meta = sm.tile([P, mb], mybir.dt.uint8, tag="meta")
                nc.sync.dma_start(meta[:], io["meta"][w * P:(w + 1) * P, :])
                si = meta[:, 0:i16b].bitcast(i16)
                drl = meta[:, i16b:i16b + 2 * T].bitcast(f16)

                g = gp.tile([P, T, HID], f16, tag="g", bufs=4)
                for o, n in chunks:
                    nc.gpsimd.dma_gather(g[:, o // P:(o + n) // P, :], xl_all[:],
                                         si[:, o // 16:(o + n) // 16],
                                         num_idxs=n, num_idxs_reg=regs[n],
                                         elem_size=HID, transpose=False)

                S = sb.tile([P, T, P], f16, tag="S", bufs=3)
                nc.vector.tensor_tensor(
                    out=S[:],
                    in0=drl[:].rearrange("p (t o) -> p t o", o=1).to_broadcast([P, T, P]),
                    in1=c_["iota"][:].rearrange("p (o d) -> p o d", o=1).to_broadcast([P, T, P]),
                    op=OP.is_equal)

                sT = sb.tile([P, T, P], f16, tag="sT")
                db = sb.tile([P, ew], f16, tag="db", bufs=1)
                nc.sync.dma_start(db[:], io["drow"][w:w + 1, :].to_broadcast([P, ew]))
                nc.vector.tensor_scalar(
                    sT[:].rearrange("p t e -> p (t e)"), db[:],
                    c_["iotat"][:], None, OP.is_equal)
                tiles[w] = dict(g=g, S=S, sT=sT)

            # ---------------- stage B: window it-1 ----------------
            # processed in two half-window slices so the cross-engine chain
            # (v -> prelu -> logits -> exp -> gw -> agg) pipelines at half-
            # window granularity
            if 1 <= it <= WIN:
                w = it - 1
                t_ = tiles[w]
                g, S, sT = t_["g"], t_["S"], t_["sT"]

                lr = sb.tile([P, T, HID], f16, tag="lr", bufs=1)
                la = sb.tile([P, T, HID], f16, tag="la", bufs=1)
                lg = sb.tile([P, T, heads], f16, tag="lg")
                ex = sb.tile([P, T, heads], f16, tag="ex")
                ad = pap.tile([P, 512], f32, tag="ad")
                dn = pap.tile([P, heads], f32, tag="dn")
                TH = (T + 1) // 2
                for h0 in range(0, T, TH):
                    h1 = min(h0 + TH, T)
                    for b0 in range(h0, h1, BATCH_M):
                        b1 = min(b0 + BATCH_M, h1)
                        pm = pmp.tile([P, BATCH_M, HID], f32, tag="pm")
                        # keep each slot's 2-matmul group consecutive:
                        # start=True wipes the whole bank's has_written bits
                        for t in range(b0, b1):
                            nc.tensor.matmul(pm[:, t - b0, :], lhsT=sT[:, t, :],
                                             rhs=xr_sb[:, w, :], start=True, stop=False)
                            nc.tensor.matmul(pm[:, t - b0, :], lhsT=c_["ident"][:],
                                             rhs=g[:, t, :], start=False, stop=True)
                        nc.scalar.activation(lr[:, b0:b1, :], pm[:, 0:b1 - b0, :],
                                             AF.Prelu, alpha=NEG)

                    nc.vector.tensor_tensor(
                        out=la[:, h0:h1, :], in0=lr[:, h0:h1, :],
                        in1=attbt[:, h0:h1, :], op=OP.mult)
                    with nc.allow_low_precision("f16 logit reduce; verified 5.9e-3"):
                        nc.vector.tensor_reduce(
                            out=lg[:, h0:h1, :].rearrange("p t h -> p (t h)"),
                            in_=la[:, h0:h1, :].rearrange("p t (h c) -> p (t h) c", h=heads),
                            axis=mybir.AxisListType.X, op=OP.add)
                    nc.scalar.activation(ex[:, h0:h1, :], lg[:, h0:h1, :], AF.Exp)

                    if heads == 1:
                        # cheaper to scale S (T*128 elems) than g (T*256)
                        nc.vector.tensor_tensor(
                            out=S[:, h0:h1, :], in0=S[:, h0:h1, :],
                            in1=ex[:, h0:h1, :].to_broadcast([P, h1 - h0, P]),
                            op=OP.mult)
                    else:
                        nc.vector.tensor_tensor(
                            out=g[:, h0:h1, :].rearrange("p t (h c) -> p t h c", h=heads),
                            in0=g[:, h0:h1, :].rearrange("p t (h c) -> p t h c", h=heads),
                            in1=ex[:, h0:h1, :].rearrange("p t (h o) -> p t h o", o=1).to_broadcast(
                                [P, h1 - h0, heads, CW]),
                            op=OP.mult)
                    # agg and den keep their groups open across all T tiles;
                    # they live in different banks (start=True wipes bank bits)
                    for t in range(h0, h1):
                        nc.tensor.matmul(ad[:, 0:HID], lhsT=S[:, t, :], rhs=g[:, t, :],
                                         start=(t == 0), stop=(t == T - 1))
                        nc.tensor.matmul(dn[:], lhsT=S[:, t, :],
                                         rhs=ex[:, t, :] if heads > 1 else c_["onesc"][:],
                                         start=(t == 0), stop=(t == T - 1))
                t_["ad"], t_["dn"] = ad, dn


def kernel(**inputs):
    per_core, shared, batch_rows, ew = _preprocess(inputs)

    if ew not in _nc_cache:
        _nc_cache[ew] = _build(ew)
    nc = _nc_cache[ew]

    in_maps = []
    for c in range(NCORES):
        m = dict(shared)
        m.update(per_core[c])
        in_maps.append({k: np.ascontiguousarray(v) for k, v in m.items()})

    res = run_bass_kernel_spmd(nc, in_maps, core_ids=list(range(NCORES)))

    B = len(np.asarray(inputs["var_node_idx"]))
    out = np.zeros((B,), np.float32)
    for c in range(NCORES):
        rows = batch_rows[c]
        out[rows] = res.results[c]["out"][0, :len(rows)]
    return out
